# revision 1
# baseline (speedup 1.0000x reference)
"""Trainium2 Bass kernel for nn_AttentionHelper (sparse_attention) — v3.

Math (per batch b):
    n1[m,l] = exp(E[m,l]/16)                      (pure exp, no bias)
    d[l]    = sum_m (mask[m]+1e-9) * n1[m,l]
    out     = sum_m (V[c,m]*mask[m]^2) * n1[m,l] / d[l]

HW-informed design (microbenchmarked): bf16 matmuls run at ~107ns per
[128x128]x[128x512] (dual-pumped), so the PE has big headroom, while the
gpsimd/Pool engine (2.6 cyc/elem on Q7) and the DVE tree were the real
bottlenecks.  v3 therefore:
  - computes the denominator ON THE PE: d_ps[p,l] += w1rep[:,j,:]^T @ n1[:,j,:]
    where w1rep[:,j,:] is w1 replicated across 128 columns — the matmul both
    reduces over m and broadcasts d to all partitions (replacing the DVE
    tree + gpsimd partition_all_reduce entirely).
  - folds mask^2 into vt AFTER the xbar transpose via 16 per-partition-scalar
    DVE muls (no partition_broadcast, no [128,L] mask tiles).
  - moves f32->bf16 conversions off the critical queues: k,v on Pool (idle
    now), q on DVE, emitted after the j-loop with a 2-slot prefetch lead.
  - AV runs cg-sequential (cg0 done mid-loop, normalized+stored while cg1
    accumulates) so PSUM fits: e_ps 2x2 + o_ps 1x2 + d_ps 1x2 = 8 banks.

Pipeline: flat (rep, batch, half) jobs; prev carried across batch AND rep
boundaries; all DMA (loads, stores, transposes) on the SP HWDGE queue —
A/B-measured ~35us/rep faster than putting stores on the ACT queue, whose
DGE kicks stall the activation pipeline.
"""

import numpy as np

import concourse.bacc as bacc
import concourse.bass as bass
import concourse.tile as tile
from concourse import mybir
from concourse.bass_utils import run_bass_kernel_spmd

B, C, L = 16, 256, 2048
NCORES = 8
BS = B // NCORES
P = 128
CCH = C // P
MCH = L // P
NH = 2
LH = L // NH
LT = 512
F32 = mybir.dt.float32
BF16 = mybir.dt.bfloat16
import os as _os
EXP = mybir.ActivationFunctionType.Exp


def _emit(ctx, tc, q_d, k_d, v_d, m_d, o_d):
    nc = tc.nc

    qk_pool = ctx.enter_context(tc.tile_pool(name="qk", bufs=2))
    stage_pool = ctx.enter_context(tc.tile_pool(name="stage", bufs=2))
    vt_pool = ctx.enter_context(tc.tile_pool(name="vt", bufs=3))
    vbf_pool = ctx.enter_context(tc.tile_pool(name="vbf", bufs=2))
    maskS_pool = ctx.enter_context(tc.tile_pool(name="maskS", bufs=3))
    w1r_pool = ctx.enter_context(tc.tile_pool(name="w1r", bufs=3))
    n1_pool = ctx.enter_context(tc.tile_pool(name="n1", bufs=2))
    out_pool = ctx.enter_context(tc.tile_pool(name="outp", bufs=2))
    rd_pool = ctx.enter_context(tc.tile_pool(name="rd", bufs=2))
    ps_e = ctx.enter_context(tc.tile_pool(name="ps_e", bufs=2, space="PSUM"))
    ps_o = ctx.enter_context(tc.tile_pool(name="ps_o", bufs=1, space="PSUM"))
    ps_d = ctx.enter_context(tc.tile_pool(name="ps_d", bufs=1, space="PSUM"))

    const_pool = ctx.enter_context(tc.tile_pool(name="const", bufs=1))
    onesP = const_pool.tile([P, P], BF16, name="onesP")
    nc.vector.memset(onesP[:], 1.0)

    state = {}

    def prep_loads(key, b):
        """DMA loads only (SP queue) — conversions happen in prep_conv."""
        st = {}
        for cc in range(CCH):
            for src, pfx in ((k_d, "k"), (q_d, "q")):
                stg = stage_pool.tile(
                    [P, L], F32, tag="stage", name=f"stg_{pfx}{key}_{cc}"
                )
                nc.sync.dma_start(out=stg[:], in_=src[b, cc * P : (cc + 1) * P, :])
                st[f"stg_{pfx}{cc}"] = stg
        mask_pt = maskS_pool.tile([P, MCH], F32, tag="mask_pt", name=f"mpt{key}")
        nc.sync.dma_start(
            out=mask_pt[:], in_=m_d[b, 0, :].rearrange("(j p) -> p j", p=P)
        )
        st["mask_pt"] = mask_pt
        for cc in range(CCH):
            vstg = stage_pool.tile([P, L], F32, tag="stage", name=f"stg_v{key}_{cc}")
            nc.sync.dma_start(out=vstg[:], in_=v_d[b, cc * P : (cc + 1) * P, :])
            st[f"stg_v{cc}"] = vstg
        state[key] = st

    def prep_conv(key):
        """Conversions + mask-derived tiles.  Emitted post-jloop so they sit
        behind the current slot's engine work, one full slot before use."""
        st = state[key]
        q_sb, k_sb = [], []
        for cc in range(CCH):
            kt = qk_pool.tile([P, L], BF16, tag=f"k{cc}", name=f"k{key}_{cc}")
            nc.gpsimd.tensor_copy(kt[:], st[f"stg_k{cc}"][:])
            k_sb.append(kt)
            qt = qk_pool.tile([P, L], BF16, tag=f"q{cc}", name=f"q{key}_{cc}")
            nc.vector.tensor_copy(qt[:], st[f"stg_q{cc}"][:])
            q_sb.append(qt)
            vb = vbf_pool.tile([P, L], BF16, tag="vbf", name=f"vbf{key}_{cc}")
            nc.gpsimd.tensor_copy(vb[:], st[f"stg_v{cc}"][:])
            st[f"v_bf{cc}"] = vb
        st["q"], st["k"] = q_sb, k_sb

        mask_pt = st["mask_pt"]
        w1 = maskS_pool.tile([P, MCH], F32, tag="w1", name=f"w1_{key}")
        nc.vector.tensor_scalar_add(w1[:], mask_pt[:], 1e-9)
        m2 = maskS_pool.tile([P, MCH], F32, tag="m2", name=f"m2_{key}")
        nc.vector.tensor_mul(m2[:], mask_pt[:], mask_pt[:])
        st["m2"] = m2
        # w1rep[:, j, c] = w1[:, j] for all c — stationary operand of the
        # denominator matmuls
        w1rep = w1r_pool.tile([P, MCH, P], BF16, tag="w1rep", name=f"w1r{key}")
        for j in range(MCH):
            nc.vector.tensor_scalar_mul(w1rep[:, j, :], onesP[:], w1[:, j : j + 1])
        st["w1rep"] = w1rep

    def prep_b(key):
        """vt transposes (SP queue; data ready) + in-place mask^2 fold."""
        st = state[key]
        vt = vt_pool.tile([P, MCH, C], BF16, tag="vt", name=f"vt{key}")
        for cc in range(CCH):
            nc.sync.dma_start_transpose(
                out=vt[:, :, cc * P : (cc + 1) * P], in_=st[f"v_bf{cc}"][:]
            )
        for j in range(MCH):
            nc.vector.tensor_scalar_mul(vt[:, j, :], vt[:, j, :], st["m2"][:, j : j + 1])
        st["vt"] = vt

    def den(prev):
        """Reciprocal of the PE-computed replicated denominator."""
        pkey, ph, pn1, pd_ps = prev
        rec = rd_pool.tile([P, LH], F32, tag="rec", name=f"rc_{pkey}_{ph}")
        nc.vector.reciprocal_approx_fast(out=rec[:], in_=pd_ps[:])
        return rec

    def jloop(cur, prev, rec):
        """QK + exp + denominator-matmuls for `cur`; AV matmuls of `prev`
        interleaved cg-sequentially with inline normalize+store per cg."""
        n1 = d_ps = None
        if cur is not None:
            key, h = cur
            st = state[key]
            lq = h * LH
            n1 = n1_pool.tile([P, MCH, LH], BF16, tag="n1", name=f"n1_{key}_{h}")
            d_ps = ps_d.tile([P, LH], F32, tag="D", name=f"d_{key}_{h}")
        av = []
        if prev is not None:
            pkey, ph, pn1, _ = prev
            pst = state[pkey]
            av = [(cg, j, lt) for cg in range(CCH) for j in range(MCH) for lt in range(2)]
        o_ps = {}

        def emit_av(k0, k1):
            for cg, j, lt in av[k0:k1]:
                if lt == 0 and j == 0:
                    o_ps[cg] = ps_o.tile(
                        [P, LH], F32, tag="O", name=f"o_{pkey}_{ph}_{cg}"
                    )
                nc.tensor.matmul(
                    o_ps[cg][:, lt * LT : (lt + 1) * LT],
                    lhsT=pst["vt"][:, j, cg * P : (cg + 1) * P],
                    rhs=pn1[:, j, lt * LT : (lt + 1) * LT],
                    start=(j == 0),
                    stop=(j == MCH - 1),
                )
                if lt == 1 and j == MCH - 1:
                    fin_cg(cg)

        def fin_cg(cg):
            out_t = out_pool.tile([P, LH], F32, tag="out", name=f"ot_{pkey}_{ph}_{cg}")
            nc.vector.tensor_mul(out_t[:], o_ps[cg][:], rec[:])
            nc.sync.dma_start(
                out=o_d[pkey[1], cg * P : (cg + 1) * P, ph * LH : (ph + 1) * LH],
                in_=out_t[:],
            )

        for j in range(MCH):
            if cur is not None:
                e_ps = ps_e.tile([P, LH], F32, tag="E", name=f"e_{key}_{h}_{j}")
                for cc in range(CCH):
                    for lt in range(2):
                        nc.tensor.matmul(
                            e_ps[:, lt * LT : (lt + 1) * LT],
                            lhsT=st["k"][cc][:, j * P : (j + 1) * P],
                            rhs=st["q"][cc][:, lq + lt * LT : lq + (lt + 1) * LT],
                            start=(cc == 0),
                            stop=(cc == CCH - 1),
                        )
                nc.scalar.activation(
                    out=n1[:, j, :], in_=e_ps[:], func=EXP, scale=1.0 / 16.0
                )
                if j > 0:
                    emit_dmm(st, n1, d_ps, j - 1)
            emit_av(4 * j, 4 * (j + 1))
        if cur is not None:
            emit_dmm(st, n1, d_ps, MCH - 1)
        emit_av(4 * MCH, len(av))
        return n1, d_ps

    def emit_dmm(st, n1, d_ps, j):
        for lt in range(2):
            nc.tensor.matmul(
                d_ps[:, lt * LT : (lt + 1) * LT],
                lhsT=st["w1rep"][:, j, :],
                rhs=n1[:, j, lt * LT : (lt + 1) * LT],
                start=(j == 0),
                stop=(j == MCH - 1),
            )

    reps = int(_os.environ.get("BASS_REPS", "1"))
    jobs = [(r, b, h) for r in range(reps) for b in range(BS) for h in range(NH)]
    prep_loads((0, jobs[0][1]), jobs[0][1])
    prep_conv((0, jobs[0][1]))
    prep_b((0, jobs[0][1]))
    prev = None
    for i, (r, b, h) in enumerate(jobs):
        rec = den(prev) if prev is not None else None
        pending_conv = None
        if h == 0 and i + 2 < len(jobs) and jobs[i + 2][2] == 0:
            nr, nb, _ = jobs[i + 2]
            prep_loads((nr, nb), nb)
            pending_conv = (nr, nb)
        elif h == 1 and i + 1 < len(jobs) and jobs[i + 1][2] == 0:
            prep_b((jobs[i + 1][0], jobs[i + 1][1]))
        n1, d_ps = jloop(((r, b), h), prev, rec)
        if pending_conv is not None:
            prep_conv(pending_conv)
        prev = ((r, b), h, n1, d_ps)
    rec = den(prev)
    jloop(None, prev, rec)


def _build():
    nc = bacc.Bacc(
        "TRN2",
        target_bir_lowering=False,
        debug=False,
        enable_asserts=False,
        num_devices=NCORES,
    )
    q_d = nc.dram_tensor("proj_query", [BS, C, L], F32, kind="ExternalInput")
    k_d = nc.dram_tensor("proj_key", [BS, C, L], F32, kind="ExternalInput")
    v_d = nc.dram_tensor("proj_val", [BS, C, L], F32, kind="ExternalInput")
    m_d = nc.dram_tensor("padding_mask", [BS, 1, L], F32, kind="ExternalInput")
    o_d = nc.dram_tensor("out", [BS, C, L], F32, kind="ExternalOutput")

    from contextlib import ExitStack

    with tile.TileContext(nc) as tc:
        with ExitStack() as ctx:
            _emit(ctx, tc, q_d.ap(), k_d.ap(), v_d.ap(), m_d.ap(), o_d.ap())
    nc.compile()
    return nc


_cached_nc = None


def get_nc():
    global _cached_nc
    if _cached_nc is None:
        _cached_nc = _build()
    return _cached_nc


def make_in_maps(proj_query, proj_key, proj_val, padding_mask):
    q = np.ascontiguousarray(np.asarray(proj_query, dtype=np.float32))
    k = np.ascontiguousarray(np.asarray(proj_key, dtype=np.float32))
    v = np.ascontiguousarray(np.asarray(proj_val, dtype=np.float32))
    m = np.ascontiguousarray(np.asarray(padding_mask, dtype=np.float32))
    assert q.shape == (B, C, L) and m.shape == (B, 1, L)
    in_maps = []
    for i in range(NCORES):
        sl = slice(i * BS, (i + 1) * BS)
        in_maps.append(
            {
                "proj_query": np.ascontiguousarray(q[sl]),
                "proj_key": np.ascontiguousarray(k[sl]),
                "proj_val": np.ascontiguousarray(v[sl]),
                "padding_mask": np.ascontiguousarray(m[sl]),
            }
        )
    return in_maps


def kernel(proj_query, proj_key, proj_val, padding_mask):
    nc = get_nc()
    in_maps = make_in_maps(proj_query, proj_key, proj_val, padding_mask)
    res = run_bass_kernel_spmd(nc, in_maps, core_ids=list(range(NCORES)))
    return np.concatenate([res.results[i]["out"] for i in range(NCORES)], axis=0)

